# revision 25
# baseline (speedup 1.0000x reference)
"""Trainium2 Bass kernel for AttnDecoderRNN single-step forward.

Sharding (8 NeuronCores, tensor-parallel):
  - attn_W / attn_b / encoder_outputs sharded over seq_len (4096 -> 512/core)
  - out_W / out_b sharded over vocab (50257 -> pad 51200 -> 6400/core)
  - combine + GRU replicated on every core (their weights stream in during
    the dead time before the first collective, so replication is free and
    removes a whole AllGather round for h_new)

All matvecs run on the TensorEngine with M=1 (batch) and the big weight
matrix as the *streaming* rhs operand, so weight bytes flow through the PE
at 1 col/cycle with only a 1-column LDWEIGHTS.  Biases are folded in as a
K=1 matmul with a ones lhsT.

Collectives: the runtime arms collectives ~55-60us into every execution,
so the critical path packs ALL pre-collective work (weight streaming,
attention scores, local softmax stats, the *unscaled* attention partial
and the full gh = h@W_hh.T) under that wall, then needs only TWO rounds:
  1. one AllGather of [neg-max, sumexp, partial(1024)] per core -- the
     softmax merge and the rank-sum of partials (a K=8 matvec on the PE)
     happen locally afterwards,
  2. one AllGather of the per-core log-softmax stats at the very end.
Log-softmax chunk stats are computed online while out_W streams.
"""

import sys

sys.path.insert(0, "/opt/trn_rl_repo")

import numpy as np

import concourse.bass as bass
import concourse.mybir as mybir
from concourse import bacc, tile

F32 = mybir.dt.float32
F32R = mybir.dt.float32r
BF16 = mybir.dt.bfloat16

NCORES = 8
I_SZ, H_SZ, V_SZ, L_SZ = 300, 1024, 50257, 4096
G3 = 3 * H_SZ               # 3072 gate rows
LSH = L_SZ // NCORES        # 512 seq positions per core
KA_T = 11                   # attn contraction tiles (11*128 = 1408 >= 1324)
VI = 6400                   # vocab shard per core (padded)
VPAD = VI * NCORES          # 51200
NCH = 13                    # stage-D chunks per core
CHS = [512] * 12 + [256]    # chunk widths (sum = 6400)
PAY = 1032                  # AG payload floats per core: m, s, 6 pad, 1024
NEG_BIG = -1.0e30
RG = [list(range(NCORES))]

MODE = "bf16"  # one of: f32, f32r, mixed, bf16


def _dtypes(mode):
    if mode == "bf16":
        return BF16, BF16
    if mode == "mixed":
        return F32, BF16
    return F32, F32


def build(mode=MODE):
    """Build the SPMD Bass program (same program on all 8 cores)."""
    adt, odt = _dtypes(mode)           # storage dtype: attn/gru weights, out_W
    use_f32r = mode in ("f32r", "mixed")

    def WA(ap):  # matmul-operand wrapper for the attn/gru path
        return ap.bitcast(F32R) if (use_f32r and adt == F32) else ap

    def WO(ap):  # matmul-operand wrapper for the output-projection path
        return ap.bitcast(F32R) if (use_f32r and odt == F32) else ap

    def RK(ap):  # rank-sum matvec operands: speed up f32 via f32r
        if ap.dtype != F32 or mode == "f32":
            return ap
        return ap.bitcast(F32R)

    nc = bacc.Bacc("TRN2", target_bir_lowering=False, debug=False,
                   num_devices=NCORES)

    def cast_dma(out, in_, casting):
        # dtype casts must use SWDGE (gpsimd); everything else small goes on
        # the ACT HWDGE ring so it never queues behind the big weight DMAs
        if casting:
            nc.gpsimd.dma_start(out=out, in_=in_)
        else:
            nc.scalar.dma_start(out=out, in_=in_)

    # ---------------- external inputs (host pre-shaped / pre-permuted) ----
    d_vattn = nc.dram_tensor("v_attn", [128, KA_T], adt, kind="ExternalInput")
    d_aw = nc.dram_tensor("aw", [128, KA_T, LSH], adt, kind="ExternalInput")
    d_ab = nc.dram_tensor("ab", [LSH], adt, kind="ExternalInput")
    d_enc = nc.dram_tensor("enc", [128, 4, H_SZ], adt, kind="ExternalInput")
    d_emb = nc.dram_tensor("embk", [128, 3], adt, kind="ExternalInput")
    d_cw = nc.dram_tensor("cw", [128, KA_T, I_SZ], adt, kind="ExternalInput")
    d_cb = nc.dram_tensor("cb", [I_SZ], adt, kind="ExternalInput")
    d_h = nc.dram_tensor("h_full", [128, 8], adt, kind="ExternalInput")
    d_hmy = nc.dram_tensor("hmyk", [128, 8], F32, kind="ExternalInput")
    d_wih = nc.dram_tensor("wih", [128, 3, G3], adt, kind="ExternalInput")
    d_whh = nc.dram_tensor("whh", [128, 8, G3], adt, kind="ExternalInput")
    d_bih = nc.dram_tensor("bih", [G3], adt, kind="ExternalInput")
    d_bhh = nc.dram_tensor("bhh", [G3], adt, kind="ExternalInput")
    d_wout_a = nc.dram_tensor("wout_a", [12, 128, 8, 512], odt,
                              kind="ExternalInput")
    d_wout_b = nc.dram_tensor("wout_b", [128, 8, 256], odt,
                              kind="ExternalInput")
    d_outb = nc.dram_tensor("outb", [VI], odt, kind="ExternalInput")

    # ---------------- external outputs ------------------------------------
    d_out_lp = nc.dram_tensor("out_logp", [NCH, 512], F32,
                              kind="ExternalOutput")
    d_out_h = nc.dram_tensor("out_h", [H_SZ], F32, kind="ExternalOutput")
    d_out_aw = nc.dram_tensor("out_attnw", [LSH], F32, kind="ExternalOutput")

    X = mybir.AxisListType.X
    ADD = mybir.AluOpType.add
    MIN = mybir.AluOpType.min
    AF = mybir.ActivationFunctionType

    wout_bufs = 4
    with tile.TileContext(nc) as tc:
        with (
            tc.tile_pool(name="w", bufs=1) as wp,
            tc.tile_pool(name="wout", bufs=wout_bufs) as wop,
            tc.tile_pool(name="ch", bufs=2) as chp,
            tc.tile_pool(name="ps", bufs=1, space="PSUM") as pp,
            tc.tile_pool(name="dram", bufs=1, space="DRAM") as dp,
        ):
            # ---- collective bounce buffers in DRAM ----
            # payload per rank: [neg-max, sumexp] kept as raw f32 bytes in
            # the header + the attention partial in the attn dtype
            pdt = adt
            HDR = 8 if pdt == F32 else 16
            PAYT = HDR + H_SZ
            cc_ab_in = dp.tile([PAYT], pdt, name="cc_ab_in")
            cc_ab_out = dp.tile([PAYT * NCORES], pdt, addr_space="Shared",
                                name="cc_ab_out")
            cc_d_in = dp.tile([8], F32, name="cc_d_in")
            cc_d_out = dp.tile([8 * NCORES], F32, addr_space="Shared",
                               name="cc_d_out")

            # ---- constants ----
            ones_a = wp.tile([1, 1], adt, name="ones_a")
            nc.vector.memset(ones_a[:, :], 1.0)
            ones_o = wp.tile([1, 1], odt, name="ones_o")
            nc.vector.memset(ones_o[:, :], 1.0)
            ones13 = wp.tile([1, NCH], F32, name="ones13")
            nc.vector.memset(ones13[:, :], 1.0)
            logits = wp.tile([NCH, 512], F32, name="logits")
            nc.vector.memset(logits[:, :], NEG_BIG)
            x_sb = wp.tile([1, 384], adt, name="x_sb")
            nc.vector.memset(x_sb[:, :], 0.0)
            ms_a = wp.tile([1, 8], F32, name="ms_a")
            nc.vector.memset(ms_a[:, :], 0.0)
            ms_d = wp.tile([1, 8], F32, name="ms_d")
            nc.vector.memset(ms_d[:, :], 0.0)

            # preload ACT function tables off the critical path (each first
            # use of a function group costs a ~1.3us ACT_TABLE_LOAD)
            pre = wp.tile([1, 1], F32, name="pre")
            nc.vector.memset(pre[:, :], 1.0)
            AFp = mybir.ActivationFunctionType
            for fn in (AFp.Exp, AFp.Sigmoid, AFp.Tanh, AFp.Ln, AFp.Relu):
                nc.scalar.activation(pre[0:1, :], pre[0:1, :], fn)

            # ---- weight / vector loads (priority order) ----
            vk = wp.tile([128, KA_T], adt, name="vk")
            nc.sync.dma_start(out=vk[:, :], in_=d_vattn[:, :])
            aw_sb = wp.tile([128, KA_T, LSH], adt, name="aw_sb")
            nc.sync.dma_start(out=aw_sb[:, :, :], in_=d_aw[:, :, :])
            ab_sb = wp.tile([1, LSH], adt, name="ab_sb")
            nc.sync.dma_start(out=ab_sb[0:1, :], in_=d_ab[:])
            enc_sb = wp.tile([128, 4, H_SZ], adt, name="enc_sb")
            nc.sync.dma_start(out=enc_sb[:, :, :], in_=d_enc[:, :, :])
            hk = wp.tile([128, 8], adt, name="hk")
            nc.sync.dma_start(out=hk[:, :], in_=d_h[:, :])
            whh_sb = wp.tile([128, 8, G3], adt, name="whh_sb")
            nc.sync.dma_start(out=whh_sb[:, :, :], in_=d_whh[:, :, :])
            bhh_sb = wp.tile([1, G3], adt, name="bhh_sb")
            nc.sync.dma_start(out=bhh_sb[0:1, :], in_=d_bhh[:])
            comb_e = wp.tile([128, 3], adt, name="comb_e")
            nc.sync.dma_start(out=comb_e[:, :], in_=d_emb[:, :])
            cw_sb = wp.tile([128, KA_T, I_SZ], adt, name="cw_sb")
            nc.sync.dma_start(out=cw_sb[:, :, :], in_=d_cw[:, :, :])
            cb_sb = wp.tile([1, I_SZ], adt, name="cb_sb")
            nc.sync.dma_start(out=cb_sb[0:1, :], in_=d_cb[:])
            wih_sb = wp.tile([128, 3, G3], adt, name="wih_sb")
            nc.sync.dma_start(out=wih_sb[:, :, :], in_=d_wih[:, :, :])
            bih_sb = wp.tile([1, G3], adt, name="bih_sb")
            nc.sync.dma_start(out=bih_sb[0:1, :], in_=d_bih[:])
            hmy_k = wp.tile([128, 8], F32, name="hmy_k")
            nc.sync.dma_start(out=hmy_k[:, :], in_=d_hmy[:, :])
            outb_sb = wp.tile([1, VI], odt, name="outb_sb")
            nc.sync.dma_start(out=outb_sb[0:1, :], in_=d_outb[:])

            # ============= stage A: attention scores + local stats ========
            ps_sc = pp.tile([1, LSH], F32, name="ps_sc")
            for t in range(KA_T):
                nc.tensor.matmul(ps_sc[0:1, :], WA(vk[:, t:t + 1]),
                                 WA(aw_sb[:, t, :]),
                                 start=(t == 0), stop=False)
            nc.tensor.matmul(ps_sc[0:1, :], WA(ones_a[:, :]),
                             WA(ab_sb[0:1, :]), start=False, stop=True)

            # stats land directly in the AG payload head: [-max, sumexp]
            nc.vector.reduce_max(ms_a[0:1, 0:1], ps_sc[0:1, :], X,
                                 negate=True)
            e_loc = wp.tile([1, LSH], F32, name="e_loc")
            nc.scalar.activation(e_loc[0:1, :], ps_sc[0:1, :], AF.Exp,
                                 bias=ms_a[0:1, 0:1], scale=1.0,
                                 accum_out=ms_a[0:1, 1:2])
            ms_src = ms_a if pdt == F32 else ms_a.bitcast(BF16)
            nc.scalar.dma_start(out=cc_ab_in[0:HDR], in_=ms_src[0:1, :])

            # ---- relayout e_loc to partition layout ----------------------
            if adt == F32:
                e_cast = e_loc
            else:
                e_cast = wp.tile([1, LSH], adt, name="e_cast")
                nc.scalar.copy(e_cast[0:1, :], e_loc[0:1, :])
            ek = wp.tile([128, 4], adt, name="ek")
            nc.scalar.dma_start(out=ek[:, :], in_=e_cast[0:1, :])

            # ============= stage B: unscaled partial attn_applied =========
            ps_att = pp.tile([1, H_SZ], F32, name="ps_att")
            for nb in range(2):
                sl = slice(nb * 512, (nb + 1) * 512)
                for t in range(4):
                    nc.tensor.matmul(ps_att[0:1, sl], WA(ek[:, t:t + 1]),
                                     WA(enc_sb[:, t, sl]),
                                     start=(t == 0), stop=(t == 3))
            attp = wp.tile([1, H_SZ], pdt, name="attp")
            nc.scalar.copy(attp[0:1, :], ps_att[0:1, :])
            nc.scalar.dma_start(out=cc_ab_in[HDR:PAYT], in_=attp[0:1, :])
            nc.gpsimd.collective_compute(
                "AllGather", mybir.AluOpType.bypass, replica_groups=RG,
                ins=[cc_ab_in[:]], outs=[cc_ab_out[:]])

            # ---- gh = h @ W_hh.T + b_hh (runs under the collective) ------
            ps_gh = pp.tile([1, 512], F32, name="ps_gh")
            gh_sb = wp.tile([1, G3], F32, name="gh_sb")
            for c in range(6):
                sl = slice(c * 512, (c + 1) * 512)
                for t in range(8):
                    nc.tensor.matmul(ps_gh[0:1, :], WA(hk[:, t:t + 1]),
                                     WA(whh_sb[:, t, sl]),
                                     start=(t == 0), stop=False)
                nc.tensor.matmul(ps_gh[0:1, :], WA(ones_a[:, :]),
                                 WA(bhh_sb[0:1, sl]), start=False, stop=True)
                nc.scalar.copy(gh_sb[0:1, sl], ps_gh[0:1, :])
            gh_k = wp.tile([128, 3, 8], F32, name="gh_k")
            for g in range(3):
                nc.scalar.dma_start(
                    out=gh_k[:, g, :],
                    in_=gh_sb[0:1, g * H_SZ:(g + 1) * H_SZ])

            # ---- AG#1 result: softmax merge + rank-sum of partials -------
            ms8_a = wp.tile([1, HDR * NCORES], pdt, name="ms8_a")
            nc.scalar.dma_start(
                out=ms8_a[0:1, :],
                in_=cc_ab_out.rearrange("(r k) -> r k", k=PAYT)[:, 0:HDR])
            ms8_f = ms8_a if pdt == F32 else ms8_a.bitcast(F32)
            ms8_av = ms8_f.rearrange("p (r k) -> p r k", k=8)
            parts = wp.tile([NCORES, PAYT], pdt, name="parts")
            nc.scalar.dma_start(
                out=parts[:, :],
                in_=cc_ab_out.rearrange("(r k) -> r k", k=PAYT))
            nmG_a = wp.tile([1, 1], F32, name="nmG_a")   # -global max
            nc.vector.tensor_reduce(nmG_a[:, :], ms8_av[:, :, 0], X, MIN)
            corr_a = wp.tile([1, NCORES], F32, name="corr_a")
            nc.scalar.activation(corr_a[0:1, :], ms8_av[:, :, 0], AF.Exp,
                                 bias=nmG_a[:, :], scale=-1.0)
            sc_a = wp.tile([1, NCORES], F32, name="sc_a")
            nc.vector.tensor_mul(sc_a[0:1, :], corr_a[0:1, :],
                                 ms8_av[:, :, 1])
            S_a = wp.tile([1, 1], F32, name="S_a")
            nc.vector.tensor_reduce(S_a[:, :], sc_a[0:1, :], X, ADD)
            rS_a = wp.tile([1, 1], F32, name="rS_a")
            nc.vector.reciprocal(rS_a[:, :], S_a[:, :])
            sc8 = wp.tile([1, NCORES], F32, name="sc8")  # exp(m_r-M)/S
            nc.vector.tensor_scalar_mul(sc8[0:1, :], corr_a[0:1, :],
                                        rS_a[:, :])
            sc8c = wp.tile([1, NCORES], pdt, name="sc8c")
            nc.scalar.copy(sc8c[0:1, :], sc8[0:1, :])
            sc8k = wp.tile([NCORES, 1], pdt, name="sc8k")
            nc.scalar.dma_start(out=sc8k[:, 0:1], in_=sc8c[0:1, :])

            # attention weights output slice (off critical path)
            cme_a = wp.tile([1, 1], F32, name="cme_a")
            nc.scalar.activation(cme_a[0:1, :], ms_a[0:1, 0:1], AF.Exp,
                                 bias=nmG_a[:, :], scale=-1.0)
            scme = wp.tile([1, 1], F32, name="scme")
            nc.vector.tensor_mul(scme[:, :], cme_a[:, :], rS_a[:, :])
            w_loc = wp.tile([1, LSH], F32, name="w_loc")
            nc.scalar.activation(w_loc[0:1, :], e_loc[0:1, :], AF.Copy,
                                 bias=0.0, scale=scme[:, :])
            nc.scalar.dma_start(out=d_out_aw[:], in_=w_loc[0:1, :])

            # rank-sum: attn_applied = sum_r sc8[r] * partial_r  (K=8 PE)
            for nb in range(2):
                sl = slice(HDR + nb * 512, HDR + (nb + 1) * 512)
                osl = slice(nb * 512, (nb + 1) * 512)
                nc.tensor.matmul(ps_att[0:1, osl], RK(sc8k[:, 0:1]),
                                 RK(parts[:, sl]), start=True, stop=True)
            att_sb = wp.tile([1, H_SZ], adt, name="att_sb")
            nc.scalar.copy(att_sb[0:1, :], ps_att[0:1, :])
            comb_a = wp.tile([128, 8], adt, name="comb_a")
            nc.scalar.dma_start(out=comb_a[:, :], in_=att_sb[0:1, :])

            # ============= stage C: combine + full GRU ====================
            ps_x = pp.tile([1, 512], F32, name="ps_x")
            for t in range(3):
                nc.tensor.matmul(ps_x[0:1, 0:I_SZ], WA(comb_e[:, t:t + 1]),
                                 WA(cw_sb[:, t, :]),
                                 start=(t == 0), stop=False)
            for t in range(3, KA_T):
                nc.tensor.matmul(ps_x[0:1, 0:I_SZ],
                                 WA(comb_a[:, t - 3:t - 2]),
                                 WA(cw_sb[:, t, :]),
                                 start=False, stop=False)
            nc.tensor.matmul(ps_x[0:1, 0:I_SZ], WA(ones_a[:, :]),
                             WA(cb_sb[0:1, :]), start=False, stop=True)
            nc.scalar.activation(x_sb[0:1, 0:I_SZ], ps_x[0:1, 0:I_SZ],
                                 AF.Relu)

            xk = wp.tile([128, 3], adt, name="xk")
            nc.scalar.dma_start(out=xk[:, :], in_=x_sb[0:1, :])

            # gi = x @ W_ih.T + b_ih over six 512-chunks (ping-pong banks)
            gi_sb = wp.tile([1, G3], F32, name="gi_sb")
            for c in range(6):
                sl = slice(c * 512, (c + 1) * 512)
                psc = ps_gh if (c % 2 == 0) else ps_x
                for t in range(3):
                    nc.tensor.matmul(psc[0:1, :], WA(xk[:, t:t + 1]),
                                     WA(wih_sb[:, t, sl]),
                                     start=(t == 0), stop=False)
                nc.tensor.matmul(psc[0:1, :], WA(ones_a[:, :]),
                                 WA(bih_sb[0:1, sl]), start=False, stop=True)
                nc.scalar.copy(gi_sb[0:1, sl], psc[0:1, :])
            gi_k = wp.tile([128, 3, 8], F32, name="gi_k")
            for g in range(3):
                nc.scalar.dma_start(
                    out=gi_k[:, g, :],
                    in_=gi_sb[0:1, g * H_SZ:(g + 1) * H_SZ])

            # gates on [128, x] layout: r,z = sigmoid(gi+gh); n = tanh(...)
            rz_in = wp.tile([128, 16], F32, name="rz_in")
            nc.vector.tensor_add(rz_in[:, :],
                                 gi_k.rearrange("p g f -> p (g f)")[:, 0:16],
                                 gh_k.rearrange("p g f -> p (g f)")[:, 0:16])
            rz = wp.tile([128, 16], F32, name="rz")
            nc.scalar.activation(rz[:, :], rz_in[:, :], AF.Sigmoid)
            rn = wp.tile([128, 8], F32, name="rn")
            nc.vector.tensor_mul(rn[:, :], rz[:, 0:8], gh_k[:, 2, :])
            n_in = wp.tile([128, 8], F32, name="n_in")
            nc.vector.tensor_add(n_in[:, :], gi_k[:, 2, :], rn[:, :])
            n_t = wp.tile([128, 8], F32, name="n_t")
            nc.scalar.activation(n_t[:, :], n_in[:, :], AF.Tanh)
            d_tl = wp.tile([128, 8], F32, name="d_tl")
            nc.vector.tensor_sub(d_tl[:, :], hmy_k[:, :], n_t[:, :])
            zd = wp.tile([128, 8], F32, name="zd")
            nc.vector.tensor_mul(zd[:, :], rz[:, 8:16], d_tl[:, :])
            hnew_k = wp.tile([128, 8], F32, name="hnew_k")
            nc.vector.tensor_add(hnew_k[:, :], n_t[:, :], zd[:, :])

            nc.scalar.dma_start(out=d_out_h[:], in_=hnew_k[:, :])
            if odt == F32:
                hnk = hnew_k
            else:
                hnk = wp.tile([128, 8], odt, name="hnk")
                nc.scalar.copy(hnk[:, :], hnew_k[:, :])

            # ---- PE warm-up: the gates phase leaves the PE idle long
            # enough for HAM to re-throttle to 1.2 GHz; a short dummy
            # matmul stream here (runs during the gates' DVE/ACT work)
            # keeps it at 2.4 GHz so stage D issues at full rate ----
            for w in range(18):
                nc.tensor.matmul(ps_sc[0:1, :], WA(ones_a[:, :]),
                                 WA(ab_sb[0:1, :]),
                                 start=True, stop=True)

            # ============= stage D: logits + online log_softmax stats =====
            mrow = wp.tile([1, 16], F32, name="mrow")   # -chunk maxes
            srow = wp.tile([1, 16], F32, name="srow")   # chunk sumexp
            for j in range(NCH):
                n_j = CHS[j]
                wt = wop.tile([128, 8, 512], odt, tag="wt", name="wt")
                if j < 12:
                    nc.sync.dma_start(out=wt[:, :, :],
                                      in_=d_wout_a[j, :, :, :])
                else:
                    nc.sync.dma_start(out=wt[:, :, 0:256],
                                      in_=d_wout_b[:, :, :])
                ps_d = pp.tile([1, 512], F32, tag="ps_d", name="ps_d",
                               bufs=2)
                for t in range(8):
                    nc.tensor.matmul(ps_d[0:1, 0:n_j],
                                     WO(hnk[:, t:t + 1]),
                                     WO(wt[:, t, 0:n_j]),
                                     start=(t == 0), stop=False)
                nc.tensor.matmul(ps_d[0:1, 0:n_j], WO(ones_o[:, :]),
                                 WO(outb_sb[0:1, j * 512:j * 512 + n_j]),
                                 start=False, stop=True)
                ch = chp.tile([1, 512], F32, tag="ch", name="ch")
                nc.scalar.copy(ch[0:1, 0:n_j], ps_d[0:1, 0:n_j])
                nc.scalar.dma_start(out=logits[j:j + 1, 0:n_j],
                                    in_=ch[0:1, 0:n_j])
                nc.vector.reduce_max(mrow[0:1, j:j + 1], ch[0:1, 0:n_j], X,
                                     negate=True)
                e_ch = chp.tile([1, 512], F32, tag="e_ch", name="e_ch")
                nc.scalar.activation(e_ch[0:1, 0:n_j], ch[0:1, 0:n_j],
                                     AF.Exp, bias=mrow[0:1, j:j + 1],
                                     scale=1.0,
                                     accum_out=srow[0:1, j:j + 1])

            # ---- merge the 13 per-chunk stats ----------------------------
            nc.vector.tensor_reduce(ms_d[0:1, 0:1], mrow[0:1, 0:NCH], X,
                                    MIN)
            corr_d = wp.tile([1, NCH], F32, name="corr_d")
            nc.scalar.activation(corr_d[0:1, :], mrow[0:1, 0:NCH], AF.Exp,
                                 bias=ms_d[0:1, 0:1], scale=-1.0)
            scd = wp.tile([1, NCH], F32, name="scd")
            nc.vector.tensor_mul(scd[0:1, :], corr_d[0:1, :],
                                 srow[0:1, 0:NCH])
            nc.vector.tensor_reduce(ms_d[0:1, 1:2], scd[0:1, :], X, ADD)
            nc.scalar.dma_start(out=cc_d_in[:], in_=ms_d[0:1, :])
            nc.gpsimd.collective_compute(
                "AllGather", mybir.AluOpType.bypass, replica_groups=RG,
                ins=[cc_d_in[:]], outs=[cc_d_out[:]])

            ms8_d = wp.tile([1, 8 * NCORES], F32, name="ms8_d")
            nc.scalar.dma_start(out=ms8_d[0:1, :], in_=cc_d_out[:])
            ms8_dv = ms8_d.rearrange("p (r k) -> p r k", k=8)
            nmG_d = wp.tile([1, 1], F32, name="nmG_d")   # -global max
            nc.vector.tensor_reduce(nmG_d[:, :], ms8_dv[:, :, 0], X, MIN)
            corr_g = wp.tile([1, NCORES], F32, name="corr_g")
            nc.scalar.activation(corr_g[0:1, :], ms8_dv[:, :, 0], AF.Exp,
                                 bias=nmG_d[:, :], scale=-1.0)
            sc_g = wp.tile([1, NCORES], F32, name="sc_g")
            nc.vector.tensor_mul(sc_g[0:1, :], corr_g[0:1, :],
                                 ms8_dv[:, :, 1])
            S_g = wp.tile([1, 1], F32, name="S_g")
            nc.vector.tensor_reduce(S_g[:, :], sc_g[0:1, :], X, ADD)
            lnS = wp.tile([1, 1], F32, name="lnS")
            nc.scalar.activation(lnS[0:1, :], S_g[0:1, :], AF.Ln)
            nshift = wp.tile([1, 1], F32, name="nshift")  # -(M + ln S)
            nc.vector.tensor_sub(nshift[:, :], nmG_d[:, :], lnS[:, :])
            nsh13 = wp.tile([1, NCH], F32, name="nsh13")
            nc.vector.tensor_scalar_mul(nsh13[0:1, :], ones13[0:1, :],
                                        nshift[:, :])
            nb13 = wp.tile([NCH, 1], F32, name="nb13")
            nc.scalar.dma_start(out=nb13[:, 0:1], in_=nsh13[0:1, :])

            outlp = wp.tile([NCH, 512], F32, name="outlp")
            nc.scalar.activation(outlp[:, :], logits[:, :], AF.Identity,
                                 bias=nb13[:, :], scale=1.0)
            nc.scalar.dma_start(out=d_out_lp[:, :], in_=outlp[:, :])

    nc.compile()
    return nc


def prepare_in_maps(embedded, hidden, encoder_outputs, attn_W, attn_b,
                    combine_W, combine_b, W_ih, W_hh, b_ih, b_hh, out_W,
                    out_b, mode=MODE):
    adt, odt = _dtypes(mode)
    anp = mybir.dt.np(adt)
    onp = mybir.dt.np(odt)

    f32 = np.float32
    emb = np.asarray(embedded, f32).reshape(I_SZ)
    h0 = np.asarray(hidden, f32).reshape(H_SZ)
    enc = np.asarray(encoder_outputs, f32)
    aW = np.asarray(attn_W, f32)
    ab = np.asarray(attn_b, f32)
    cW = np.asarray(combine_W, f32)
    cb = np.asarray(combine_b, f32)
    Wih = np.asarray(W_ih, f32)
    Whh = np.asarray(W_hh, f32)
    bih = np.asarray(b_ih, f32)
    bhh = np.asarray(b_hh, f32)
    oW = np.asarray(out_W, f32)
    ob = np.asarray(out_b, f32)

    v = np.zeros(128 * KA_T, f32)
    v[:I_SZ] = emb
    v[I_SZ:I_SZ + H_SZ] = h0
    v_attn = v.reshape(128, KA_T).astype(anp)

    emb_pad = np.zeros(384, f32)
    emb_pad[:I_SZ] = emb
    embk = emb_pad.reshape(128, 3).astype(anp)
    h_full = h0.reshape(128, 8).astype(anp)
    hmyk = h0.reshape(128, 8).astype(f32)

    AWT = aW.T  # [1324, 4096]
    CWT = cW.T  # [1324, 300]
    cw_e = np.zeros((384, I_SZ), f32)
    cw_e[:I_SZ] = CWT[:I_SZ]
    cw_host = np.concatenate(
        [cw_e.reshape(128, 3, I_SZ), CWT[I_SZ:].reshape(128, 8, I_SZ)],
        axis=1).astype(anp)  # [128, 11, 300]

    wih_p = np.zeros((384, G3), f32)
    wih_p[:I_SZ] = Wih.T
    wih_host = wih_p.reshape(128, 3, G3).astype(anp)
    whh_host = Whh.T.reshape(128, 8, G3).astype(anp)

    WTp = np.zeros((H_SZ, VPAD), f32)
    WTp[:, :V_SZ] = oW.T
    obp = np.full(VPAD, NEG_BIG, f32)
    obp[:V_SZ] = ob

    in_maps = []
    for c in range(NCORES):
        AWc = np.zeros((128 * KA_T, LSH), f32)
        AWc[:I_SZ + H_SZ] = AWT[:, c * LSH:(c + 1) * LSH]
        Wc = WTp[:, c * VI:(c + 1) * VI].reshape(128, 8, VI)
        in_maps.append({
            "v_attn": v_attn,
            "aw": AWc.reshape(128, KA_T, LSH).astype(anp),
            "ab": ab[c * LSH:(c + 1) * LSH].astype(anp),
            "enc": enc[c * LSH:(c + 1) * LSH].reshape(128, 4, H_SZ)
                   .astype(anp).copy(),
            "embk": embk,
            "cw": cw_host,
            "cb": cb.astype(anp),
            "h_full": h_full,
            "hmyk": hmyk,
            "wih": wih_host,
            "whh": whh_host,
            "bih": bih.astype(anp),
            "bhh": bhh.astype(anp),
            "wout_a": Wc[:, :, :6144].reshape(128, 8, 12, 512)
                      .transpose(2, 0, 1, 3).astype(onp).copy(),
            "wout_b": Wc[:, :, 6144:].astype(onp).copy(),
            "outb": obp[c * VI:(c + 1) * VI].astype(onp),
        })
    return in_maps


def gather_outputs(results):
    """results: list of 8 dicts with out_logp/out_h/out_attnw."""
    lp_parts = []
    for c in range(NCORES):
        r = np.asarray(results[c]["out_logp"], np.float32).reshape(NCH, 512)
        lp_parts.append(r[:12].reshape(-1))
        lp_parts.append(r[12, :256])
    output = np.concatenate(lp_parts)[:V_SZ][None, :]
    h_new = np.asarray(results[0]["out_h"],
                       np.float32).reshape(-1)[None, None, :]
    attn_w = np.concatenate(
        [np.asarray(results[c]["out_attnw"], np.float32).reshape(-1)
         for c in range(NCORES)])[None, :]
    return output, h_new, attn_w


_NC_CACHE = {}


def kernel(embedded, hidden, encoder_outputs, attn_W, attn_b,
           combine_W, combine_b, W_ih, W_hh, b_ih, b_hh, out_W, out_b):
    from concourse.bass_utils import run_bass_kernel_spmd

    if MODE not in _NC_CACHE:
        _NC_CACHE[MODE] = build(MODE)
    nc = _NC_CACHE[MODE]
    in_maps = prepare_in_maps(embedded, hidden, encoder_outputs, attn_W,
                              attn_b, combine_W, combine_b, W_ih, W_hh,
                              b_ih, b_hh, out_W, out_b, mode=MODE)
    res = run_bass_kernel_spmd(nc, in_maps, list(range(NCORES)))
    return gather_outputs(res.results)


# revision 26
# speedup vs baseline: 1.2091x; 1.2091x over previous
"""Trainium2 Bass kernel for AttnDecoderRNN single-step forward.

Sharding (8 NeuronCores, tensor-parallel):
  - attn_W / attn_b / encoder_outputs sharded over seq_len (4096 -> 512/core)
  - out_W / out_b sharded over vocab (50257 -> pad 51200 -> 6400/core)
  - combine + GRU replicated on every core (their weights stream in during
    the dead time before the first collective, so replication is free and
    removes a whole AllGather round for h_new)

All matvecs run on the TensorEngine with M=1 (batch) and the big weight
matrix as the *streaming* rhs operand, so weight bytes flow through the PE
at 1 col/cycle with only a 1-column LDWEIGHTS.  Biases are folded in as a
K=1 matmul with a ones lhsT.

Collectives: the runtime arms collectives ~55-60us into every execution,
so the critical path packs ALL pre-collective work (weight streaming,
attention scores, local softmax stats, the *unscaled* attention partial
and the full gh = h@W_hh.T) under that wall, then needs only TWO rounds:
  1. one AllGather of [neg-max, sumexp, partial(1024)] per core -- the
     softmax merge and the rank-sum of partials (a K=8 matvec on the PE)
     happen locally afterwards,
  2. one AllGather of the per-core log-softmax stats at the very end.
Log-softmax chunk stats are computed online while out_W streams.
"""

import sys

sys.path.insert(0, "/opt/trn_rl_repo")

import numpy as np

import concourse.bass as bass
import concourse.mybir as mybir
from concourse import bacc, tile

F32 = mybir.dt.float32
F32R = mybir.dt.float32r
BF16 = mybir.dt.bfloat16

NCORES = 8
I_SZ, H_SZ, V_SZ, L_SZ = 300, 1024, 50257, 4096
G3 = 3 * H_SZ               # 3072 gate rows
LSH = L_SZ // NCORES        # 512 seq positions per core
KA_T = 11                   # attn contraction tiles (11*128 = 1408 >= 1324)
VI = 6400                   # vocab shard per core (padded)
VPAD = VI * NCORES          # 51200
NCH = 13                    # stage-D chunks per core
CHS = [512] * 12 + [256]    # chunk widths (sum = 6400)
PAY = 1032                  # AG payload floats per core: m, s, 6 pad, 1024
NEG_BIG = -1.0e30
RG = [list(range(NCORES))]

MODE = "bf16"  # one of: f32, f32r, mixed, bf16


def _dtypes(mode):
    if mode == "bf16":
        return BF16, BF16
    if mode == "mixed":
        return F32, BF16
    return F32, F32


def build(mode=MODE):
    """Build the SPMD Bass program (same program on all 8 cores)."""
    adt, odt = _dtypes(mode)           # storage dtype: attn/gru weights, out_W
    use_f32r = mode in ("f32r", "mixed")

    def WA(ap):  # matmul-operand wrapper for the attn/gru path
        return ap.bitcast(F32R) if (use_f32r and adt == F32) else ap

    def WO(ap):  # matmul-operand wrapper for the output-projection path
        return ap.bitcast(F32R) if (use_f32r and odt == F32) else ap

    def RK(ap):  # rank-sum matvec operands: speed up f32 via f32r
        if ap.dtype != F32 or mode == "f32":
            return ap
        return ap.bitcast(F32R)

    nc = bacc.Bacc("TRN2", target_bir_lowering=False, debug=False,
                   num_devices=NCORES)

    def cast_dma(out, in_, casting):
        # dtype casts must use SWDGE (gpsimd); everything else small goes on
        # the ACT HWDGE ring so it never queues behind the big weight DMAs
        if casting:
            nc.gpsimd.dma_start(out=out, in_=in_)
        else:
            nc.scalar.dma_start(out=out, in_=in_)

    # ---------------- external inputs (host pre-shaped / pre-permuted) ----
    d_vattn = nc.dram_tensor("v_attn", [128, KA_T], adt, kind="ExternalInput")
    d_aw = nc.dram_tensor("aw", [128, KA_T, LSH], adt, kind="ExternalInput")
    d_ab = nc.dram_tensor("ab", [LSH], adt, kind="ExternalInput")
    d_enc = nc.dram_tensor("enc", [128, 4, H_SZ], adt, kind="ExternalInput")
    d_emb = nc.dram_tensor("embk", [128, 3], adt, kind="ExternalInput")
    d_cw = nc.dram_tensor("cw", [128, KA_T, I_SZ], adt, kind="ExternalInput")
    d_cb = nc.dram_tensor("cb", [I_SZ], adt, kind="ExternalInput")
    d_h = nc.dram_tensor("h_full", [128, 8], adt, kind="ExternalInput")
    d_hmy = nc.dram_tensor("hmyk", [128, 8], F32, kind="ExternalInput")
    d_wih = nc.dram_tensor("wih", [128, 3, G3], adt, kind="ExternalInput")
    d_whh = nc.dram_tensor("whh", [128, 8, G3], adt, kind="ExternalInput")
    d_bih = nc.dram_tensor("bih", [G3], adt, kind="ExternalInput")
    d_bhh = nc.dram_tensor("bhh", [G3], adt, kind="ExternalInput")
    d_wout_a = nc.dram_tensor("wout_a", [12, 128, 8, 512], odt,
                              kind="ExternalInput")
    d_wout_b = nc.dram_tensor("wout_b", [128, 8, 256], odt,
                              kind="ExternalInput")
    d_outb = nc.dram_tensor("outb", [VI], odt, kind="ExternalInput")

    # ---------------- external outputs ------------------------------------
    d_out_lp = nc.dram_tensor("out_logp", [NCH, 512], F32,
                              kind="ExternalOutput")
    d_out_h = nc.dram_tensor("out_h", [H_SZ], F32, kind="ExternalOutput")
    d_out_aw = nc.dram_tensor("out_attnw", [LSH], F32, kind="ExternalOutput")

    X = mybir.AxisListType.X
    ADD = mybir.AluOpType.add
    MIN = mybir.AluOpType.min
    AF = mybir.ActivationFunctionType

    wout_bufs = 4
    with tile.TileContext(nc) as tc:
        with (
            tc.tile_pool(name="w", bufs=1) as wp,
            tc.tile_pool(name="wout", bufs=wout_bufs) as wop,
            tc.tile_pool(name="ch", bufs=2) as chp,
            tc.tile_pool(name="ps", bufs=1, space="PSUM") as pp,
            tc.tile_pool(name="dram", bufs=1, space="DRAM") as dp,
        ):
            # ---- collective bounce buffers in DRAM ----
            cc_ab_in = dp.tile([PAY], F32, name="cc_ab_in")
            cc_ab_out = dp.tile([PAY * NCORES], F32, addr_space="Shared",
                                name="cc_ab_out")
            cc_d_in = dp.tile([8], F32, name="cc_d_in")
            cc_d_out = dp.tile([8 * NCORES], F32, addr_space="Shared",
                               name="cc_d_out")

            # ---- constants ----
            ones_a = wp.tile([1, 1], adt, name="ones_a")
            nc.vector.memset(ones_a[:, :], 1.0)
            ones_o = wp.tile([1, 1], odt, name="ones_o")
            nc.vector.memset(ones_o[:, :], 1.0)
            ones13 = wp.tile([1, NCH], F32, name="ones13")
            nc.vector.memset(ones13[:, :], 1.0)
            logits = wp.tile([NCH, 512], F32, name="logits")
            nc.vector.memset(logits[:, :], NEG_BIG)
            x_sb = wp.tile([1, 384], adt, name="x_sb")
            nc.vector.memset(x_sb[:, :], 0.0)
            ms_a = wp.tile([1, 8], F32, name="ms_a")
            nc.vector.memset(ms_a[:, :], 0.0)
            ms_d = wp.tile([1, 8], F32, name="ms_d")
            nc.vector.memset(ms_d[:, :], 0.0)

            # preload ACT function tables off the critical path (each first
            # use of a function group costs a ~1.3us ACT_TABLE_LOAD)
            pre = wp.tile([1, 1], F32, name="pre")
            nc.vector.memset(pre[:, :], 1.0)
            AFp = mybir.ActivationFunctionType
            for fn in (AFp.Exp, AFp.Sigmoid, AFp.Tanh, AFp.Ln, AFp.Relu):
                nc.scalar.activation(pre[0:1, :], pre[0:1, :], fn)

            # ---- weight / vector loads (priority order) ----
            vk = wp.tile([128, KA_T], adt, name="vk")
            nc.sync.dma_start(out=vk[:, :], in_=d_vattn[:, :])
            aw_sb = wp.tile([128, KA_T, LSH], adt, name="aw_sb")
            nc.sync.dma_start(out=aw_sb[:, :, :], in_=d_aw[:, :, :])
            ab_sb = wp.tile([1, LSH], adt, name="ab_sb")
            nc.sync.dma_start(out=ab_sb[0:1, :], in_=d_ab[:])
            enc_sb = wp.tile([128, 4, H_SZ], adt, name="enc_sb")
            nc.sync.dma_start(out=enc_sb[:, :, :], in_=d_enc[:, :, :])
            hk = wp.tile([128, 8], adt, name="hk")
            nc.sync.dma_start(out=hk[:, :], in_=d_h[:, :])
            whh_sb = wp.tile([128, 8, G3], adt, name="whh_sb")
            nc.sync.dma_start(out=whh_sb[:, :, :], in_=d_whh[:, :, :])
            bhh_sb = wp.tile([1, G3], adt, name="bhh_sb")
            nc.sync.dma_start(out=bhh_sb[0:1, :], in_=d_bhh[:])
            comb_e = wp.tile([128, 3], adt, name="comb_e")
            nc.sync.dma_start(out=comb_e[:, :], in_=d_emb[:, :])
            cw_sb = wp.tile([128, KA_T, I_SZ], adt, name="cw_sb")
            nc.sync.dma_start(out=cw_sb[:, :, :], in_=d_cw[:, :, :])
            cb_sb = wp.tile([1, I_SZ], adt, name="cb_sb")
            nc.sync.dma_start(out=cb_sb[0:1, :], in_=d_cb[:])
            wih_sb = wp.tile([128, 3, G3], adt, name="wih_sb")
            nc.sync.dma_start(out=wih_sb[:, :, :], in_=d_wih[:, :, :])
            bih_sb = wp.tile([1, G3], adt, name="bih_sb")
            nc.sync.dma_start(out=bih_sb[0:1, :], in_=d_bih[:])
            hmy_k = wp.tile([128, 8], F32, name="hmy_k")
            nc.sync.dma_start(out=hmy_k[:, :], in_=d_hmy[:, :])
            outb_sb = wp.tile([1, VI], odt, name="outb_sb")
            nc.sync.dma_start(out=outb_sb[0:1, :], in_=d_outb[:])

            # ============= stage A: attention scores + local stats ========
            ps_sc = pp.tile([1, LSH], F32, name="ps_sc")
            for t in range(KA_T):
                nc.tensor.matmul(ps_sc[0:1, :], WA(vk[:, t:t + 1]),
                                 WA(aw_sb[:, t, :]),
                                 start=(t == 0), stop=False)
            nc.tensor.matmul(ps_sc[0:1, :], WA(ones_a[:, :]),
                             WA(ab_sb[0:1, :]), start=False, stop=True)

            # stats land directly in the AG payload head: [-max, sumexp]
            nc.vector.reduce_max(ms_a[0:1, 0:1], ps_sc[0:1, :], X,
                                 negate=True)
            e_loc = wp.tile([1, LSH], F32, name="e_loc")
            nc.scalar.activation(e_loc[0:1, :], ps_sc[0:1, :], AF.Exp,
                                 bias=ms_a[0:1, 0:1], scale=1.0,
                                 accum_out=ms_a[0:1, 1:2])
            nc.scalar.dma_start(out=cc_ab_in[0:8], in_=ms_a[0:1, :])

            # ---- relayout e_loc to partition layout ----------------------
            if adt == F32:
                e_cast = e_loc
            else:
                e_cast = wp.tile([1, LSH], adt, name="e_cast")
                nc.scalar.copy(e_cast[0:1, :], e_loc[0:1, :])
            ek = wp.tile([128, 4], adt, name="ek")
            nc.scalar.dma_start(out=ek[:, :], in_=e_cast[0:1, :])

            # ============= stage B: unscaled partial attn_applied =========
            ps_att = pp.tile([1, H_SZ], F32, name="ps_att")
            for nb in range(2):
                sl = slice(nb * 512, (nb + 1) * 512)
                for t in range(4):
                    nc.tensor.matmul(ps_att[0:1, sl], WA(ek[:, t:t + 1]),
                                     WA(enc_sb[:, t, sl]),
                                     start=(t == 0), stop=(t == 3))
            attp = wp.tile([1, H_SZ], F32, name="attp")
            nc.scalar.copy(attp[0:1, :], ps_att[0:1, :])
            nc.scalar.dma_start(out=cc_ab_in[8:PAY], in_=attp[0:1, :])
            nc.gpsimd.collective_compute(
                "AllGather", mybir.AluOpType.bypass, replica_groups=RG,
                ins=[cc_ab_in[:]], outs=[cc_ab_out[:]])

            # ---- gh = h @ W_hh.T + b_hh (runs under the collective) ------
            ps_gh = pp.tile([1, 512], F32, name="ps_gh")
            gh_sb = wp.tile([1, G3], F32, name="gh_sb")
            for c in range(6):
                sl = slice(c * 512, (c + 1) * 512)
                for t in range(8):
                    nc.tensor.matmul(ps_gh[0:1, :], WA(hk[:, t:t + 1]),
                                     WA(whh_sb[:, t, sl]),
                                     start=(t == 0), stop=False)
                nc.tensor.matmul(ps_gh[0:1, :], WA(ones_a[:, :]),
                                 WA(bhh_sb[0:1, sl]), start=False, stop=True)
                nc.scalar.copy(gh_sb[0:1, sl], ps_gh[0:1, :])
            gh_k = wp.tile([128, 3, 8], F32, name="gh_k")
            for g in range(3):
                nc.scalar.dma_start(
                    out=gh_k[:, g, :],
                    in_=gh_sb[0:1, g * H_SZ:(g + 1) * H_SZ])

            # ---- AG#1 result: softmax merge + rank-sum of partials -------
            ms8_a = wp.tile([1, 2 * NCORES], F32, name="ms8_a")
            nc.scalar.dma_start(
                out=ms8_a[0:1, :],
                in_=cc_ab_out.rearrange("(r k) -> r k", k=PAY)[:, 0:2])
            ms8_av = ms8_a.rearrange("p (r k) -> p r k", k=2)
            parts = wp.tile([NCORES, PAY], F32, name="parts")
            nc.scalar.dma_start(
                out=parts[:, :],
                in_=cc_ab_out.rearrange("(r k) -> r k", k=PAY))
            nmG_a = wp.tile([1, 1], F32, name="nmG_a")   # -global max
            nc.vector.tensor_reduce(nmG_a[:, :], ms8_av[:, :, 0], X, MIN)
            corr_a = wp.tile([1, NCORES], F32, name="corr_a")
            nc.scalar.activation(corr_a[0:1, :], ms8_av[:, :, 0], AF.Exp,
                                 bias=nmG_a[:, :], scale=-1.0)
            sc_a = wp.tile([1, NCORES], F32, name="sc_a")
            nc.vector.tensor_mul(sc_a[0:1, :], corr_a[0:1, :],
                                 ms8_av[:, :, 1])
            S_a = wp.tile([1, 1], F32, name="S_a")
            nc.vector.tensor_reduce(S_a[:, :], sc_a[0:1, :], X, ADD)
            rS_a = wp.tile([1, 1], F32, name="rS_a")
            nc.vector.reciprocal(rS_a[:, :], S_a[:, :])
            sc8 = wp.tile([1, NCORES], F32, name="sc8")  # exp(m_r-M)/S
            nc.vector.tensor_scalar_mul(sc8[0:1, :], corr_a[0:1, :],
                                        rS_a[:, :])
            sc8k = wp.tile([NCORES, 1], F32, name="sc8k")
            nc.scalar.dma_start(out=sc8k[:, 0:1], in_=sc8[0:1, :])

            # attention weights output slice (off critical path)
            cme_a = wp.tile([1, 1], F32, name="cme_a")
            nc.scalar.activation(cme_a[0:1, :], ms_a[0:1, 0:1], AF.Exp,
                                 bias=nmG_a[:, :], scale=-1.0)
            scme = wp.tile([1, 1], F32, name="scme")
            nc.vector.tensor_mul(scme[:, :], cme_a[:, :], rS_a[:, :])
            w_loc = wp.tile([1, LSH], F32, name="w_loc")
            nc.scalar.activation(w_loc[0:1, :], e_loc[0:1, :], AF.Copy,
                                 bias=0.0, scale=scme[:, :])
            nc.scalar.dma_start(out=d_out_aw[:], in_=w_loc[0:1, :])

            # rank-sum: attn_applied = sum_r sc8[r] * partial_r  (K=8 PE)
            for nb in range(2):
                sl = slice(8 + nb * 512, 8 + (nb + 1) * 512)
                osl = slice(nb * 512, (nb + 1) * 512)
                nc.tensor.matmul(ps_att[0:1, osl], RK(sc8k[:, 0:1]),
                                 RK(parts[:, sl]), start=True, stop=True)
            att_sb = wp.tile([1, H_SZ], adt, name="att_sb")
            nc.scalar.copy(att_sb[0:1, :], ps_att[0:1, :])
            comb_a = wp.tile([128, 8], adt, name="comb_a")
            nc.scalar.dma_start(out=comb_a[:, :], in_=att_sb[0:1, :])

            # ============= stage C: combine + full GRU ====================
            ps_x = pp.tile([1, 512], F32, name="ps_x")
            for t in range(3):
                nc.tensor.matmul(ps_x[0:1, 0:I_SZ], WA(comb_e[:, t:t + 1]),
                                 WA(cw_sb[:, t, :]),
                                 start=(t == 0), stop=False)
            for t in range(3, KA_T):
                nc.tensor.matmul(ps_x[0:1, 0:I_SZ],
                                 WA(comb_a[:, t - 3:t - 2]),
                                 WA(cw_sb[:, t, :]),
                                 start=False, stop=False)
            nc.tensor.matmul(ps_x[0:1, 0:I_SZ], WA(ones_a[:, :]),
                             WA(cb_sb[0:1, :]), start=False, stop=True)
            nc.scalar.activation(x_sb[0:1, 0:I_SZ], ps_x[0:1, 0:I_SZ],
                                 AF.Relu)

            xk = wp.tile([128, 3], adt, name="xk")
            nc.scalar.dma_start(out=xk[:, :], in_=x_sb[0:1, :])

            # gi = x @ W_ih.T + b_ih over six 512-chunks (ping-pong banks)
            gi_sb = wp.tile([1, G3], F32, name="gi_sb")
            for c in range(6):
                sl = slice(c * 512, (c + 1) * 512)
                psc = ps_gh if (c % 2 == 0) else ps_x
                for t in range(3):
                    nc.tensor.matmul(psc[0:1, :], WA(xk[:, t:t + 1]),
                                     WA(wih_sb[:, t, sl]),
                                     start=(t == 0), stop=False)
                nc.tensor.matmul(psc[0:1, :], WA(ones_a[:, :]),
                                 WA(bih_sb[0:1, sl]), start=False, stop=True)
                nc.scalar.copy(gi_sb[0:1, sl], psc[0:1, :])
            gi_k = wp.tile([128, 3, 8], F32, name="gi_k")
            for g in range(3):
                nc.scalar.dma_start(
                    out=gi_k[:, g, :],
                    in_=gi_sb[0:1, g * H_SZ:(g + 1) * H_SZ])

            # gates on [128, x] layout: r,z = sigmoid(gi+gh); n = tanh(...)
            rz_in = wp.tile([128, 16], F32, name="rz_in")
            nc.vector.tensor_add(rz_in[:, :],
                                 gi_k.rearrange("p g f -> p (g f)")[:, 0:16],
                                 gh_k.rearrange("p g f -> p (g f)")[:, 0:16])
            rz = wp.tile([128, 16], F32, name="rz")
            nc.scalar.activation(rz[:, :], rz_in[:, :], AF.Sigmoid)
            rn = wp.tile([128, 8], F32, name="rn")
            nc.vector.tensor_mul(rn[:, :], rz[:, 0:8], gh_k[:, 2, :])
            n_in = wp.tile([128, 8], F32, name="n_in")
            nc.vector.tensor_add(n_in[:, :], gi_k[:, 2, :], rn[:, :])
            n_t = wp.tile([128, 8], F32, name="n_t")
            nc.scalar.activation(n_t[:, :], n_in[:, :], AF.Tanh)
            d_tl = wp.tile([128, 8], F32, name="d_tl")
            nc.vector.tensor_sub(d_tl[:, :], hmy_k[:, :], n_t[:, :])
            zd = wp.tile([128, 8], F32, name="zd")
            nc.vector.tensor_mul(zd[:, :], rz[:, 8:16], d_tl[:, :])
            hnew_k = wp.tile([128, 8], F32, name="hnew_k")
            nc.vector.tensor_add(hnew_k[:, :], n_t[:, :], zd[:, :])

            nc.scalar.dma_start(out=d_out_h[:], in_=hnew_k[:, :])
            if odt == F32:
                hnk = hnew_k
            else:
                hnk = wp.tile([128, 8], odt, name="hnk")
                nc.scalar.copy(hnk[:, :], hnew_k[:, :])

            # ---- PE warm-up: the gates phase leaves the PE idle long
            # enough for HAM to re-throttle to 1.2 GHz; a short dummy
            # matmul stream here (runs during the gates' DVE/ACT work)
            # keeps it at 2.4 GHz so stage D issues at full rate ----
            for w in range(18):
                nc.tensor.matmul(ps_sc[0:1, :], WA(ones_a[:, :]),
                                 WA(ab_sb[0:1, :]),
                                 start=True, stop=True)

            # ============= stage D: logits + online log_softmax stats =====
            mrow = wp.tile([1, 16], F32, name="mrow")   # -chunk maxes
            srow = wp.tile([1, 16], F32, name="srow")   # chunk sumexp
            for j in range(NCH):
                n_j = CHS[j]
                wt = wop.tile([128, 8, 512], odt, tag="wt", name="wt")
                if j < 12:
                    nc.sync.dma_start(out=wt[:, :, :],
                                      in_=d_wout_a[j, :, :, :])
                else:
                    nc.sync.dma_start(out=wt[:, :, 0:256],
                                      in_=d_wout_b[:, :, :])
                ps_d = pp.tile([1, 512], F32, tag="ps_d", name="ps_d",
                               bufs=2)
                for t in range(8):
                    nc.tensor.matmul(ps_d[0:1, 0:n_j],
                                     WO(hnk[:, t:t + 1]),
                                     WO(wt[:, t, 0:n_j]),
                                     start=(t == 0), stop=False)
                nc.tensor.matmul(ps_d[0:1, 0:n_j], WO(ones_o[:, :]),
                                 WO(outb_sb[0:1, j * 512:j * 512 + n_j]),
                                 start=False, stop=True)
                ch = chp.tile([1, 512], F32, tag="ch", name="ch")
                nc.scalar.copy(ch[0:1, 0:n_j], ps_d[0:1, 0:n_j])
                nc.scalar.dma_start(out=logits[j:j + 1, 0:n_j],
                                    in_=ch[0:1, 0:n_j])
                nc.vector.reduce_max(mrow[0:1, j:j + 1], ch[0:1, 0:n_j], X,
                                     negate=True)
                e_ch = chp.tile([1, 512], F32, tag="e_ch", name="e_ch")
                nc.scalar.activation(e_ch[0:1, 0:n_j], ch[0:1, 0:n_j],
                                     AF.Exp, bias=mrow[0:1, j:j + 1],
                                     scale=1.0,
                                     accum_out=srow[0:1, j:j + 1])

            # ---- merge the 13 per-chunk stats ----------------------------
            nc.vector.tensor_reduce(ms_d[0:1, 0:1], mrow[0:1, 0:NCH], X,
                                    MIN)
            corr_d = wp.tile([1, NCH], F32, name="corr_d")
            nc.scalar.activation(corr_d[0:1, :], mrow[0:1, 0:NCH], AF.Exp,
                                 bias=ms_d[0:1, 0:1], scale=-1.0)
            scd = wp.tile([1, NCH], F32, name="scd")
            nc.vector.tensor_mul(scd[0:1, :], corr_d[0:1, :],
                                 srow[0:1, 0:NCH])
            nc.vector.tensor_reduce(ms_d[0:1, 1:2], scd[0:1, :], X, ADD)
            nc.scalar.dma_start(out=cc_d_in[:], in_=ms_d[0:1, :])
            nc.gpsimd.collective_compute(
                "AllGather", mybir.AluOpType.bypass, replica_groups=RG,
                ins=[cc_d_in[:]], outs=[cc_d_out[:]])

            ms8_d = wp.tile([1, 8 * NCORES], F32, name="ms8_d")
            nc.scalar.dma_start(out=ms8_d[0:1, :], in_=cc_d_out[:])
            ms8_dv = ms8_d.rearrange("p (r k) -> p r k", k=8)
            nmG_d = wp.tile([1, 1], F32, name="nmG_d")   # -global max
            nc.vector.tensor_reduce(nmG_d[:, :], ms8_dv[:, :, 0], X, MIN)
            corr_g = wp.tile([1, NCORES], F32, name="corr_g")
            nc.scalar.activation(corr_g[0:1, :], ms8_dv[:, :, 0], AF.Exp,
                                 bias=nmG_d[:, :], scale=-1.0)
            sc_g = wp.tile([1, NCORES], F32, name="sc_g")
            nc.vector.tensor_mul(sc_g[0:1, :], corr_g[0:1, :],
                                 ms8_dv[:, :, 1])
            S_g = wp.tile([1, 1], F32, name="S_g")
            nc.vector.tensor_reduce(S_g[:, :], sc_g[0:1, :], X, ADD)
            lnS = wp.tile([1, 1], F32, name="lnS")
            nc.scalar.activation(lnS[0:1, :], S_g[0:1, :], AF.Ln)
            nshift = wp.tile([1, 1], F32, name="nshift")  # -(M + ln S)
            nc.vector.tensor_sub(nshift[:, :], nmG_d[:, :], lnS[:, :])
            nsh13 = wp.tile([1, NCH], F32, name="nsh13")
            nc.vector.tensor_scalar_mul(nsh13[0:1, :], ones13[0:1, :],
                                        nshift[:, :])
            nb13 = wp.tile([NCH, 1], F32, name="nb13")
            nc.scalar.dma_start(out=nb13[:, 0:1], in_=nsh13[0:1, :])

            outlp = wp.tile([NCH, 512], F32, name="outlp")
            nc.scalar.activation(outlp[:, :], logits[:, :], AF.Identity,
                                 bias=nb13[:, :], scale=1.0)
            nc.scalar.dma_start(out=d_out_lp[:, :], in_=outlp[:, :])

    nc.compile()
    return nc


def prepare_in_maps(embedded, hidden, encoder_outputs, attn_W, attn_b,
                    combine_W, combine_b, W_ih, W_hh, b_ih, b_hh, out_W,
                    out_b, mode=MODE):
    adt, odt = _dtypes(mode)
    anp = mybir.dt.np(adt)
    onp = mybir.dt.np(odt)

    f32 = np.float32
    emb = np.asarray(embedded, f32).reshape(I_SZ)
    h0 = np.asarray(hidden, f32).reshape(H_SZ)
    enc = np.asarray(encoder_outputs, f32)
    aW = np.asarray(attn_W, f32)
    ab = np.asarray(attn_b, f32)
    cW = np.asarray(combine_W, f32)
    cb = np.asarray(combine_b, f32)
    Wih = np.asarray(W_ih, f32)
    Whh = np.asarray(W_hh, f32)
    bih = np.asarray(b_ih, f32)
    bhh = np.asarray(b_hh, f32)
    oW = np.asarray(out_W, f32)
    ob = np.asarray(out_b, f32)

    v = np.zeros(128 * KA_T, f32)
    v[:I_SZ] = emb
    v[I_SZ:I_SZ + H_SZ] = h0
    v_attn = v.reshape(128, KA_T).astype(anp)

    emb_pad = np.zeros(384, f32)
    emb_pad[:I_SZ] = emb
    embk = emb_pad.reshape(128, 3).astype(anp)
    h_full = h0.reshape(128, 8).astype(anp)
    hmyk = h0.reshape(128, 8).astype(f32)

    AWT = aW.T  # [1324, 4096]
    CWT = cW.T  # [1324, 300]
    cw_e = np.zeros((384, I_SZ), f32)
    cw_e[:I_SZ] = CWT[:I_SZ]
    cw_host = np.concatenate(
        [cw_e.reshape(128, 3, I_SZ), CWT[I_SZ:].reshape(128, 8, I_SZ)],
        axis=1).astype(anp)  # [128, 11, 300]

    wih_p = np.zeros((384, G3), f32)
    wih_p[:I_SZ] = Wih.T
    wih_host = wih_p.reshape(128, 3, G3).astype(anp)
    whh_host = Whh.T.reshape(128, 8, G3).astype(anp)

    WTp = np.zeros((H_SZ, VPAD), f32)
    WTp[:, :V_SZ] = oW.T
    obp = np.full(VPAD, NEG_BIG, f32)
    obp[:V_SZ] = ob

    in_maps = []
    for c in range(NCORES):
        AWc = np.zeros((128 * KA_T, LSH), f32)
        AWc[:I_SZ + H_SZ] = AWT[:, c * LSH:(c + 1) * LSH]
        Wc = WTp[:, c * VI:(c + 1) * VI].reshape(128, 8, VI)
        in_maps.append({
            "v_attn": v_attn,
            "aw": AWc.reshape(128, KA_T, LSH).astype(anp),
            "ab": ab[c * LSH:(c + 1) * LSH].astype(anp),
            "enc": enc[c * LSH:(c + 1) * LSH].reshape(128, 4, H_SZ)
                   .astype(anp).copy(),
            "embk": embk,
            "cw": cw_host,
            "cb": cb.astype(anp),
            "h_full": h_full,
            "hmyk": hmyk,
            "wih": wih_host,
            "whh": whh_host,
            "bih": bih.astype(anp),
            "bhh": bhh.astype(anp),
            "wout_a": Wc[:, :, :6144].reshape(128, 8, 12, 512)
                      .transpose(2, 0, 1, 3).astype(onp).copy(),
            "wout_b": Wc[:, :, 6144:].astype(onp).copy(),
            "outb": obp[c * VI:(c + 1) * VI].astype(onp),
        })
    return in_maps


def gather_outputs(results):
    """results: list of 8 dicts with out_logp/out_h/out_attnw."""
    lp_parts = []
    for c in range(NCORES):
        r = np.asarray(results[c]["out_logp"], np.float32).reshape(NCH, 512)
        lp_parts.append(r[:12].reshape(-1))
        lp_parts.append(r[12, :256])
    output = np.concatenate(lp_parts)[:V_SZ][None, :]
    h_new = np.asarray(results[0]["out_h"],
                       np.float32).reshape(-1)[None, None, :]
    attn_w = np.concatenate(
        [np.asarray(results[c]["out_attnw"], np.float32).reshape(-1)
         for c in range(NCORES)])[None, :]
    return output, h_new, attn_w


_NC_CACHE = {}


def kernel(embedded, hidden, encoder_outputs, attn_W, attn_b,
           combine_W, combine_b, W_ih, W_hh, b_ih, b_hh, out_W, out_b):
    from concourse.bass_utils import run_bass_kernel_spmd

    if MODE not in _NC_CACHE:
        _NC_CACHE[MODE] = build(MODE)
    nc = _NC_CACHE[MODE]
    in_maps = prepare_in_maps(embedded, hidden, encoder_outputs, attn_W,
                              attn_b, combine_W, combine_b, W_ih, W_hh,
                              b_ih, b_hh, out_W, out_b, mode=MODE)
    res = run_bass_kernel_spmd(nc, in_maps, list(range(NCORES)))
    return gather_outputs(res.results)
